# revision 2
# baseline (speedup 1.0000x reference)
"""AttentionConv (7x7 windowed per-channel softmax attention) on 8 TRN2 cores.

Sharding: core = (chalf, batch, shalf); chalf=1 maps stored transposed so
rel_w folds like rel_h.  Per core: 128 channels x 28x56 positions.

Mixed-precision pipeline (vs the all-fp32 V1 at 230 us):
  Phase 1 (PE f32r, TF32-rounded inputs): q/k/v projections into fp32 PSUM;
    k stays fp32 (kpad), q -> fp16 (ACT copies), v -> fp16 (DVE copies).
  Phase 2, per window offset (d1, d2):
    kb[d1] = kpad rows + rel[:,d1] -> fp16   GpSimd broadcast-add, per d1
    s = kb_view * q                          fp16 TT on DVE (16-bit 2x mode)
    e = exp(s - 48) -> bf16                  ACT (the exp stream is the
                                             irreducible ~73 us/core floor)
    t = e * v -> bf16                        TT, 16/49 offsets on GpSimd
    den += I@e ; num += I@t                  PE bf16 identity matmuls into
                                             3x512 PSUM banks + a shared
                                             tail bank (sub-bank offsets,
                                             pre-zeroed, start=False - a
                                             start=True would reset the
                                             whole bank)
  out = num * reciprocal(den)                DVE
Cross-rep software pipelining: the next rep's projection groups + rel folds
are emitted interleaved into the current rep's phase-2 stream (DRAIN_BY),
with a static 8-bank PSUM layout (mm 1 + den 3 + num 3 + tails 1) so the
single mm bank hands off cleanly between reps.

Numerics: 9.4e-3 scale-relative absmax on HW (gate 2e-2): fp16 kb/s/q/v
rounding dominates (logit abs error ~|s|*2^-10.5), plus bf16 e/t.
Cost-model per-rep ~102 us; engines ACT/DVE/Pool/PE all at 77-84 us busy.
"""
import numpy as np
from contextlib import ExitStack

import jax
from jax.sharding import Mesh, PartitionSpec
from jax.experimental.shard_map import shard_map

import concourse.bass as bass
import concourse.bacc as bacc
import concourse.tile as tile
from concourse import mybir
from concourse import bass2jax

F32 = mybir.dt.float32
F32R = mybir.dt.float32r
BF16 = mybir.dt.bfloat16
FP16 = mybir.dt.float16

B, H, W, CIN, CO, K, PAD = 2, 56, 56, 512, 256, 7, 3
OWN = 28
SPAN = 31
PR = 34
PW = 62
NPOS = PR * 56      # 1904
NOWN = OWN * 56     # 1568
SHIFT = -48.0
NSL = 4
SLW = NOWN // NSL   # 392

_CACHE = {}
N_TP = 16           # t-mult offsets sent to GpSimd (of 49)
BUFS = 6
PROJ_F32R = True
V_FP16 = True
KB_ENG = "pool"     # engine for the per-d1 rel folds: pool | dve | act
NCH_DMA = 8         # x DMA position-chunks (each covers all 4 cin tiles)


def _build_nc(reps=1, n_tp=N_TP, proj_f32r=PROJ_F32R,
              v_fp16=V_FP16, bufs=BUFS, kb_eng=KB_ENG, nch=NCH_DMA):
    nc = bacc.Bacc("TRN2", target_bir_lowering=False, debug=False)
    XDT = F32R if proj_f32r else F32
    xt = nc.dram_tensor("xt", [CIN, NPOS], XDT, kind="ExternalInput").ap()
    wt = nc.dram_tensor("wt", [3, CIN, 128], XDT, kind="ExternalInput").ap()
    rel = nc.dram_tensor("rel", [128, K], F32, kind="ExternalInput").ap()
    ident = nc.dram_tensor("ident", [128, 128], BF16, kind="ExternalInput").ap()
    nbias = nc.dram_tensor("nbias", [128, 1], F32, kind="ExternalInput").ap()
    out = nc.dram_tensor("out", [128, NOWN], F32, kind="ExternalOutput").ap()

    VDT = FP16 if v_fp16 else BF16
    # matmul/recip col splits: 3x512 into den/num banks + 32-tail into the
    # shared dnt bank (sub-bank offsets for the two tails)
    slc = [(0, 512), (512, 512), (1024, 512)]

    # interleaved pool-assignment for the t-mult: n_tp of 49 offsets on
    # GpSimd; first/last kept on DVE so pipeline ends aren't on the slow
    # engine
    t_pool = [False] * 49
    acc = 0
    for j in range(1, 48):
        want = j * n_tp // 47
        if want > acc:
            t_pool[j] = True
            acc = want

    with tile.TileContext(nc) as tc, ExitStack() as ctx:
        per = ctx.enter_context(tc.tile_pool(name="per", bufs=1))
        ld = ctx.enter_context(tc.tile_pool(name="ld", bufs=1))

        wsb = ld.tile([128, 3, 4, 128], XDT)
        wtv = wt.rearrange("w (t p) m -> p w t m", p=128)
        nc.sync.dma_start(out=wsb[:, 1], in_=wtv[:, 1])   # k weights
        nc.sync.dma_start(out=wsb[:, 0], in_=wtv[:, 0])   # q weights
        relsb = per.tile([128, K], F32)
        nc.sync.dma_start(out=relsb, in_=rel)
        identsb = per.tile([128, 128], BF16)
        nc.sync.dma_start(out=identsb, in_=ident)
        nbsb = per.tile([128, 1], F32)
        nc.sync.dma_start(out=nbsb, in_=nbias)
        xsb = ld.tile([128, 4, NPOS], XDT)
        xtv = xt.rearrange("(t p) n -> p t n", p=128)
        chw = NPOS // nch
        for c in range(nch):
            if nch < 0:   # batched variant (t-dims in one DMA)
                nc.sync.dma_start(out=xsb[:, :, c * chw:(c + 1) * chw],
                                  in_=xtv[:, :, c * chw:(c + 1) * chw])
            else:
                for t in range(4):
                    nc.sync.dma_start(out=xsb[:, t, c * chw:(c + 1) * chw],
                                      in_=xtv[:, t, c * chw:(c + 1) * chw])
        nc.sync.dma_start(out=wsb[:, 2], in_=wtv[:, 2])   # v weights

        maps = ctx.enter_context(tc.tile_pool(name="maps", bufs=2))
        kbp = ctx.enter_context(tc.tile_pool(name="kbp", bufs=2))
        sp = ctx.enter_context(tc.tile_pool(name="sp", bufs=bufs))
        ep = ctx.enter_context(tc.tile_pool(name="ep", bufs=bufs))
        tp = ctx.enter_context(tc.tile_pool(name="tp", bufs=bufs))
        fin = ctx.enter_context(tc.tile_pool(name="fin", bufs=2))
        # static PSUM layout (8 banks total): mm 1 + den 3 + num 3 + dnt 1
        mm = ctx.enter_context(tc.tile_pool(name="mm", bufs=1, space="PSUM"))
        accp = ctx.enter_context(tc.tile_pool(name="acc", bufs=1,
                                              space="PSUM"))

        kv_slices = [(0, 7), (7, 7), (14, 7), (21, 7), (28, 6)]

        def make_p1(ri):
            """Allocate rep ri's map tiles and return (tiles, emit-closures).

            The closures are interleaved into the previous rep's phase-2
            emission so the projection matmuls fill PE gaps and the single
            mm PSUM bank never serializes back-to-back groups.
            """
            kpad = maps.tile([128, PR, PW], F32, tag="kpad", name=f"kpad{ri}")
            vpad = maps.tile([128, PR, PW], VDT, tag="vpad", name=f"vpad{ri}")
            qsb = maps.tile([128, NOWN], FP16, tag="qsb", name=f"qsb{ri}")
            kbs = [kbp.tile([128, OWN, PW], FP16, tag=f"kb{d1}",
                            name=f"kb{d1}_{ri}")
                   for d1 in range(K)]
            tiles = (kpad, vpad, qsb, kbs)

            def memsets():
                for buf in (kpad, vpad):
                    nc.gpsimd.memset(buf[:, :, 0:PAD], 0.0)
                    nc.gpsimd.memset(buf[:, :, PAD + 56:PW], 0.0)

            def proj_kv(wi, dst, r0, nr, eng_copy):
                pt = mm.tile([128, 392], F32, tag="mmkv", name="pt")
                n0, n1 = r0 * 56, (r0 + nr) * 56
                for t in range(4):
                    nc.tensor.matmul(pt[:, :nr * 56],
                                     lhsT=wsb[:, wi, t, :],
                                     rhs=xsb[:, t, n0:n1],
                                     start=(t == 0), stop=(t == 3))
                dstv = dst[:, r0:r0 + nr, PAD:PAD + 56]
                srcv = pt[:, :nr * 56].rearrange("p (r c) -> p r c", r=nr)
                if eng_copy == "act":
                    nc.scalar.copy(out=dstv, in_=srcv)
                elif eng_copy == "dma":
                    nc.sync.dma_start(out=dstv, in_=srcv)
                else:
                    nc.vector.tensor_copy(out=dstv, in_=srcv)

            def proj_q(i):
                pt = mm.tile([128, SLW], F32, tag="mmkv", name="pt")
                n0 = PAD * 56 + i * SLW
                for t in range(4):
                    nc.tensor.matmul(pt, lhsT=wsb[:, 0, t, :],
                                     rhs=xsb[:, t, n0:n0 + SLW],
                                     start=(t == 0), stop=(t == 3))
                nc.scalar.copy(out=qsb[:, i * SLW:(i + 1) * SLW], in_=pt)

            def build_kb(d1):
                dst, srcv = kbs[d1], kpad[:, d1:d1 + OWN, :]
                if kb_eng == "pool":
                    nc.gpsimd.tensor_tensor(
                        out=dst, in0=srcv,
                        in1=relsb[:, d1:d1 + 1].broadcast_to((128, OWN, PW)),
                        op=mybir.AluOpType.add)
                elif kb_eng == "act":
                    nc.scalar.activation(
                        out=dst, in_=srcv,
                        func=mybir.ActivationFunctionType.Identity,
                        bias=relsb[:, d1:d1 + 1], scale=1.0)
                else:
                    nc.vector.tensor_scalar_add(out=dst, in0=srcv,
                                                scalar1=relsb[:, d1:d1 + 1])

            from functools import partial
            items = [memsets]
            items += [partial(proj_kv, 1, kpad, r0, nr, "act")
                      for (r0, nr) in kv_slices]
            items += [partial(build_kb, d1) for d1 in range(K)]
            items += [partial(proj_q, i) for i in range(NSL)]
            items += [partial(proj_kv, 2, vpad, r0, nr, "dve")
                      for (r0, nr) in kv_slices]
            return tiles, items

        def phase2(tiles, nxt_items):
            """Emit rep's attention stream, draining nxt_items between
            offsets (all drained by offset DRAIN_BY so the next rep can
            start immediately)."""
            kpad, vpad, qsb, kbs = tiles
            den = accp.tile([128, 3, 512], F32, tag="den", name="den")
            num = accp.tile([128, 3, 512], F32, tag="num", name="num")
            dnt = accp.tile([128, 2, 32], F32, tag="dnt", name="dnt")
            nc.vector.memset(dnt, 0.0)   # start=True resets the whole bank,
            # so the two sub-bank tails accumulate with start=False onto 0

            q3 = qsb.rearrange("p (r c) -> p r c", r=OWN)
            DRAIN_BY = 45
            nit = len(nxt_items)
            drained = 0
            j = 0
            for d1 in range(K):
                for d2 in range(K):
                    want = min(nit, nit * (j + 1) // DRAIN_BY + 1)
                    while drained < want:
                        nxt_items[drained]()
                        drained += 1
                    st = sp.tile([128, OWN, 56], FP16, tag="s", name="st")
                    nc.vector.tensor_tensor(
                        out=st,
                        in0=kbs[d1][:, :, d2:d2 + 56],
                        in1=q3,
                        op=mybir.AluOpType.mult)
                    et = ep.tile([128, NOWN], BF16, tag="e", name="et")
                    nc.scalar.activation(
                        out=et.rearrange("p (r c) -> p r c", r=OWN),
                        in_=st,
                        func=mybir.ActivationFunctionType.Exp,
                        bias=nbsb, scale=1.0)
                    tt = tp.tile([128, NOWN], BF16, tag="t", name="tt")
                    eng_t = nc.gpsimd if t_pool[j] else nc.vector
                    eng_t.tensor_tensor(
                        out=tt.rearrange("p (r c) -> p r c", r=OWN),
                        in0=et.rearrange("p (r c) -> p r c", r=OWN),
                        in1=vpad[:, d1:d1 + OWN, d2:d2 + 56],
                        op=mybir.AluOpType.mult)
                    first = (d1 == 0 and d2 == 0)
                    last = (d1 == K - 1 and d2 == K - 1)
                    for i, (c0, cw) in enumerate(slc):
                        nc.tensor.matmul(
                            den[:, i, :cw], lhsT=identsb,
                            rhs=et[:, c0:c0 + cw],
                            start=first, stop=last, skip_group_check=True)
                        nc.tensor.matmul(
                            num[:, i, :cw], lhsT=identsb,
                            rhs=tt[:, c0:c0 + cw],
                            start=first, stop=last, skip_group_check=True)
                    nc.tensor.matmul(
                        dnt[:, 0, :], lhsT=identsb, rhs=et[:, 1536:NOWN],
                        start=False, stop=last, skip_group_check=True)
                    nc.tensor.matmul(
                        dnt[:, 1, :], lhsT=identsb, rhs=tt[:, 1536:NOWN],
                        start=False, stop=last, skip_group_check=True)
                    j += 1
            while drained < nit:
                nxt_items[drained]()
                drained += 1

            rden = fin.tile([128, NOWN], F32, tag="rden", name="rden")
            outsb = fin.tile([128, NOWN], F32, tag="outsb", name="outsb")
            views = [(c0, cw, den[:, i, :cw], num[:, i, :cw])
                     for i, (c0, cw) in enumerate(slc)]
            views.append((1536, 32, dnt[:, 0, :], dnt[:, 1, :]))
            for c0, cw, dv, nv in views:
                sl = slice(c0, c0 + cw)
                nc.vector.reciprocal_approx_fast(out=rden[:, sl], in_=dv)
                nc.vector.tensor_tensor(out=outsb[:, sl], in0=nv,
                                        in1=rden[:, sl],
                                        op=mybir.AluOpType.mult)
            nc.sync.dma_start(out=out, in_=outsb)

        cur_tiles, cur_items = make_p1(0)
        for it in cur_items:
            it()
        for r in range(reps):
            if r + 1 < reps:
                nxt_tiles, nxt_items = make_p1(r + 1)
            else:
                nxt_tiles, nxt_items = None, []
            phase2(cur_tiles, nxt_items)
            cur_tiles = nxt_tiles

    nc.finalize()
    return nc


def _prep_inputs(x, w_q, w_k, w_v, rel_h, rel_w):
    """Build the 8 per-core input dicts (all host-side numpy)."""
    import ml_dtypes
    x4 = np.ascontiguousarray(np.asarray(x, np.float32).reshape(B, H, W, CIN))
    relh = np.asarray(rel_h, np.float32).reshape(128, K)
    relw = np.asarray(rel_w, np.float32).reshape(128, K)
    ws = [np.asarray(w, np.float32) for w in (w_q, w_k, w_v)]
    ident = np.eye(128, dtype=ml_dtypes.bfloat16)
    nbias = np.full((128, 1), SHIFT, np.float32)

    in_maps = []
    for core in range(8):
        chalf, b, shalf = core >> 2, (core >> 1) & 1, core & 1
        if chalf == 0:
            xm = x4[b]
            rel = relh
        else:
            xm = x4[b].transpose(1, 0, 2)
            rel = relw
        arr = np.zeros((PR, 56, CIN), np.float32)
        if shalf == 0:
            arr[PAD:PAD + SPAN] = xm[0:SPAN]
        else:
            arr[0:SPAN] = xm[H - SPAN:H]
        xt = np.ascontiguousarray(arr.reshape(NPOS, CIN).T)
        cs = slice(chalf * 128, chalf * 128 + 128)
        wt = np.ascontiguousarray(
            np.stack([w[cs].T for w in ws]))
        in_maps.append({"xt": xt, "wt": wt, "rel": np.ascontiguousarray(rel),
                        "ident": ident, "nbias": nbias})
    return in_maps


def _make_runner(nc, n_cores=8):
    bass2jax.install_neuronx_cc_hook()
    in_names, out_names, out_avals = [], [], []
    partition_name = (nc.partition_id_tensor.name
                      if nc.partition_id_tensor else None)
    for alloc in nc.m.functions[0].allocations:
        if not isinstance(alloc, mybir.MemoryLocationSet):
            continue
        name = alloc.memorylocations[0].name
        if alloc.kind == "ExternalInput":
            if name != partition_name:
                in_names.append(name)
        elif alloc.kind == "ExternalOutput":
            out_names.append(name)
            shape = tuple(alloc.tensor_shape)
            dtype = mybir.dt.np(alloc.dtype)
            out_avals.append(jax.core.ShapedArray(shape, dtype))
    n_params = len(in_names)
    n_outs = len(out_names)
    all_names = list(in_names) + out_names
    if partition_name is not None:
        all_names.append(partition_name)

    def _body(*args):
        operands = list(args)
        if partition_name is not None:
            operands.append(bass2jax.partition_id_tensor())
        outs = bass2jax._bass_exec_p.bind(
            *operands, out_avals=tuple(out_avals), in_names=tuple(all_names),
            out_names=tuple(out_names), lowering_input_output_aliases=(),
            sim_require_finite=True, sim_require_nnan=True, nc=nc)
        return tuple(outs)

    devices = jax.devices()[:n_cores]
    mesh = Mesh(np.asarray(devices), ("core",))
    donate = tuple(range(n_params, n_params + n_outs))
    sharded = jax.jit(
        shard_map(_body, mesh=mesh,
                  in_specs=(PartitionSpec("core"),) * (n_params + n_outs),
                  out_specs=(PartitionSpec("core"),) * n_outs,
                  check_rep=False),
        donate_argnums=donate, keep_unused=True)
    return sharded, in_names, out_names, out_avals


def _get_compiled(reps=1, **kw):
    key = ("runner", reps, tuple(sorted(kw.items())))
    if key not in _CACHE:
        nc = _build_nc(reps=reps, **kw)
        _CACHE[key] = _make_runner(nc)
    return _CACHE[key]


def make_device_args(in_maps, reps=1, **kw):
    _, in_names, _, _ = _get_compiled(reps, **kw)
    return [np.concatenate([np.asarray(m[nm]) for m in in_maps], axis=0)
            for nm in in_names]


def run_cores(concat_in, reps=1, **kw):
    sharded, in_names, out_names, out_avals = _get_compiled(reps, **kw)
    concat_zeros = [np.zeros((8 * a.shape[0], *a.shape[1:]), a.dtype)
                    for a in out_avals]
    outs = sharded(*concat_in, *concat_zeros)
    o = np.asarray(outs[out_names.index("out")]).reshape(8, 128, NOWN)
    return o


def _assemble(per_core_out):
    out4 = np.empty((B, CO, H, W), np.float32)
    for core in range(8):
        chalf, b, shalf = core >> 2, (core >> 1) & 1, core & 1
        blk = per_core_out[core].reshape(128, OWN, 56)
        lo = shalf * OWN
        if chalf == 0:
            out4[b, 0:128, lo:lo + OWN, :] = blk
        else:
            out4[b, 128:256, :, lo:lo + OWN] = blk.transpose(0, 2, 1)
    return out4.reshape(B, CO * H, W)


def kernel(x, w_q, w_k, w_v, rel_h, rel_w):
    in_maps = _prep_inputs(x, w_q, w_k, w_v, rel_h, rel_w)
    concat_in = make_device_args(in_maps)
    per_core = run_cores(concat_in)
    return _assemble(per_core)


# revision 7
# speedup vs baseline: 1.5669x; 1.5669x over previous
"""AttentionConv (7x7 windowed per-channel softmax attention) on 8 TRN2 cores.

Sharding: core = (chalf, batch, shalf); chalf=1 maps stored transposed so
rel_w folds like rel_h.  Per core: 128 channels x 28x56 positions.

Mixed-precision pipeline (vs the all-fp32 V1 at 230 us):
  Phase 1 (PE f32r, TF32-rounded inputs): q/k/v projections into fp32 PSUM;
    k stays fp32 (kpad), q -> fp16 (ACT copies), v -> fp16 (DVE copies).
  Phase 2, per window offset (d1, d2):
    kb[d1] = kpad rows + rel[:,d1] -> fp16   GpSimd broadcast-add, per d1
    s = kb_view * q                          fp16 TT on DVE (16-bit 2x mode)
    e = exp(s - 48) -> bf16                  ACT (the exp stream is the
                                             irreducible ~73 us/core floor)
    t = e * v -> bf16                        TT, 16/49 offsets on GpSimd
    den += I@e ; num += I@t                  PE bf16 identity matmuls into
                                             3x512 PSUM banks + a shared
                                             tail bank (sub-bank offsets,
                                             pre-zeroed, start=False - a
                                             start=True would reset the
                                             whole bank)
  out = num * reciprocal(den)                DVE
Cross-rep software pipelining: the next rep's projection groups + rel folds
are emitted interleaved into the current rep's phase-2 stream (DRAIN_BY),
with a static 8-bank PSUM layout (mm 1 + den 3 + num 3 + tails 1) so the
single mm bank hands off cleanly between reps.

Numerics: 9.4e-3 scale-relative absmax on HW (gate 2e-2): fp16 kb/s/q/v
rounding dominates (logit abs error ~|s|*2^-10.5), plus bf16 e/t.
Cost-model per-rep ~102 us; engines ACT/DVE/Pool/PE all at 77-84 us busy.
"""
import numpy as np
from contextlib import ExitStack

import jax
from jax.sharding import Mesh, PartitionSpec
from jax.experimental.shard_map import shard_map

import concourse.bass as bass
import concourse.bacc as bacc
import concourse.tile as tile
from concourse import mybir
from concourse import bass2jax

F32 = mybir.dt.float32
F32R = mybir.dt.float32r
BF16 = mybir.dt.bfloat16
FP16 = mybir.dt.float16

B, H, W, CIN, CO, K, PAD = 2, 56, 56, 512, 256, 7, 3
OWN = 28
SPAN = 31
PR = 34
PW = 62
NPOS = PR * 56      # 1904
NOWN = OWN * 56     # 1568
SHIFT = -48.0
NSL = 4
SLW = NOWN // NSL   # 392

_CACHE = {}
N_TP = 16           # t-mult offsets sent to GpSimd (of 49)
BUFS = 6
PROJ_F32R = True
V_FP16 = True
KB_ENG = "pool"     # engine for the per-d1 rel folds: pool | dve | act
NCH_DMA = 8         # x DMA position-chunks (each covers all 4 cin tiles)


def _build_nc(reps=1, n_tp=N_TP, proj_f32r=PROJ_F32R,
              v_fp16=V_FP16, bufs=BUFS, kb_eng=KB_ENG, nch=NCH_DMA):
    nc = bacc.Bacc("TRN2", target_bir_lowering=False, debug=False)
    XDT = F32R if proj_f32r else F32
    xt = nc.dram_tensor("xt", [CIN, NPOS], XDT, kind="ExternalInput").ap()
    wt = nc.dram_tensor("wt", [3, CIN, 128], XDT, kind="ExternalInput").ap()
    rel = nc.dram_tensor("rel", [128, K], F32, kind="ExternalInput").ap()
    ident = nc.dram_tensor("ident", [128, 128], BF16, kind="ExternalInput").ap()
    nbias = nc.dram_tensor("nbias", [128, 1], F32, kind="ExternalInput").ap()
    out = nc.dram_tensor("out", [128, NOWN], F32, kind="ExternalOutput").ap()

    VDT = FP16 if v_fp16 else BF16
    # matmul/recip col splits: 3x512 into den/num banks + 32-tail into the
    # shared dnt bank (sub-bank offsets for the two tails)
    slc = [(0, 512), (512, 512), (1024, 512)]

    # interleaved pool-assignment for the t-mult: n_tp of 49 offsets on
    # GpSimd; first/last kept on DVE so pipeline ends aren't on the slow
    # engine
    t_pool = [False] * 49
    acc = 0
    for j in range(1, 48):
        want = j * n_tp // 47
        if want > acc:
            t_pool[j] = True
            acc = want

    with tile.TileContext(nc) as tc, ExitStack() as ctx:
        per = ctx.enter_context(tc.tile_pool(name="per", bufs=1))
        ld = ctx.enter_context(tc.tile_pool(name="ld", bufs=1))

        wsb = ld.tile([128, 3, 4, 128], XDT)
        wtv = wt.rearrange("w (t p) m -> p w t m", p=128)
        nc.sync.dma_start(out=wsb[:, 1], in_=wtv[:, 1])   # k weights
        nc.sync.dma_start(out=wsb[:, 0], in_=wtv[:, 0])   # q weights
        relsb = per.tile([128, K], F32)
        nc.sync.dma_start(out=relsb, in_=rel)
        identsb = per.tile([128, 128], BF16)
        nc.sync.dma_start(out=identsb, in_=ident)
        nbsb = per.tile([128, 1], F32)
        nc.sync.dma_start(out=nbsb, in_=nbias)
        xsb = ld.tile([128, 4, NPOS], XDT)
        xtv = xt.rearrange("(t p) n -> p t n", p=128)
        chw = NPOS // nch
        for c in range(nch):
            if nch < 0:   # batched variant (t-dims in one DMA)
                nc.sync.dma_start(out=xsb[:, :, c * chw:(c + 1) * chw],
                                  in_=xtv[:, :, c * chw:(c + 1) * chw])
            else:
                for t in range(4):
                    nc.sync.dma_start(out=xsb[:, t, c * chw:(c + 1) * chw],
                                      in_=xtv[:, t, c * chw:(c + 1) * chw])
        nc.sync.dma_start(out=wsb[:, 2], in_=wtv[:, 2])   # v weights

        maps = ctx.enter_context(tc.tile_pool(name="maps", bufs=2))
        kbp = ctx.enter_context(tc.tile_pool(name="kbp", bufs=2))
        sp = ctx.enter_context(tc.tile_pool(name="sp", bufs=bufs))
        ep = ctx.enter_context(tc.tile_pool(name="ep", bufs=bufs))
        tp = ctx.enter_context(tc.tile_pool(name="tp", bufs=bufs))
        fin = ctx.enter_context(tc.tile_pool(name="fin", bufs=2))
        # static PSUM layout (8 banks total): mm 1 + den 3 + num 3 + dnt 1
        mm = ctx.enter_context(tc.tile_pool(name="mm", bufs=1, space="PSUM"))
        accp = ctx.enter_context(tc.tile_pool(name="acc", bufs=1,
                                              space="PSUM"))

        kv_slices = [(0, 7), (7, 7), (14, 7), (21, 7), (28, 6)]

        def make_p1(ri):
            """Allocate rep ri's map tiles and return (tiles, emit-closures).

            The closures are interleaved into the previous rep's phase-2
            emission so the projection matmuls fill PE gaps and the single
            mm PSUM bank never serializes back-to-back groups.
            """
            kpad = maps.tile([128, PR, PW], F32, tag="kpad", name=f"kpad{ri}")
            vpad = maps.tile([128, PR, PW], VDT, tag="vpad", name=f"vpad{ri}")
            qsb = maps.tile([128, NOWN], FP16, tag="qsb", name=f"qsb{ri}")
            kbs = [kbp.tile([128, OWN, PW], FP16, tag=f"kb{d1}",
                            name=f"kb{d1}_{ri}")
                   for d1 in range(K)]
            tiles = (kpad, vpad, qsb, kbs)

            def memsets():
                for buf in (kpad, vpad):
                    nc.gpsimd.memset(buf[:, :, 0:PAD], 0.0)
                    nc.gpsimd.memset(buf[:, :, PAD + 56:PW], 0.0)

            def proj_kv(wi, dst, r0, nr, eng_copy):
                pt = mm.tile([128, 392], F32, tag="mmkv", name="pt")
                n0, n1 = r0 * 56, (r0 + nr) * 56
                for t in range(4):
                    nc.tensor.matmul(pt[:, :nr * 56],
                                     lhsT=wsb[:, wi, t, :],
                                     rhs=xsb[:, t, n0:n1],
                                     start=(t == 0), stop=(t == 3))
                dstv = dst[:, r0:r0 + nr, PAD:PAD + 56]
                srcv = pt[:, :nr * 56].rearrange("p (r c) -> p r c", r=nr)
                if eng_copy == "act":
                    nc.scalar.copy(out=dstv, in_=srcv)
                elif eng_copy == "dma":
                    nc.sync.dma_start(out=dstv, in_=srcv)
                else:
                    nc.vector.tensor_copy(out=dstv, in_=srcv)

            def proj_q(i):
                pt = mm.tile([128, SLW], F32, tag="mmkv", name="pt")
                n0 = PAD * 56 + i * SLW
                for t in range(4):
                    nc.tensor.matmul(pt, lhsT=wsb[:, 0, t, :],
                                     rhs=xsb[:, t, n0:n0 + SLW],
                                     start=(t == 0), stop=(t == 3))
                nc.scalar.copy(out=qsb[:, i * SLW:(i + 1) * SLW], in_=pt)

            def build_kb(d1):
                dst, srcv = kbs[d1], kpad[:, d1:d1 + OWN, :]
                if kb_eng == "pool":
                    nc.gpsimd.tensor_tensor(
                        out=dst, in0=srcv,
                        in1=relsb[:, d1:d1 + 1].broadcast_to((128, OWN, PW)),
                        op=mybir.AluOpType.add)
                elif kb_eng == "act":
                    nc.scalar.activation(
                        out=dst, in_=srcv,
                        func=mybir.ActivationFunctionType.Identity,
                        bias=relsb[:, d1:d1 + 1], scale=1.0)
                else:
                    nc.vector.tensor_scalar_add(out=dst, in0=srcv,
                                                scalar1=relsb[:, d1:d1 + 1])

            from functools import partial
            items = [memsets]
            items += [partial(proj_kv, 1, kpad, r0, nr, "act")
                      for (r0, nr) in kv_slices]
            items += [partial(build_kb, d1) for d1 in range(K)]
            items += [partial(proj_q, i) for i in range(NSL)]
            items += [partial(proj_kv, 2, vpad, r0, nr, "dve")
                      for (r0, nr) in kv_slices]
            return tiles, items

        def phase2(tiles, nxt_items):
            """Emit rep's attention stream, draining nxt_items between
            offsets (all drained by offset DRAIN_BY so the next rep can
            start immediately)."""
            kpad, vpad, qsb, kbs = tiles
            den = accp.tile([128, 3, 512], F32, tag="den", name="den")
            num = accp.tile([128, 3, 512], F32, tag="num", name="num")
            dnt = accp.tile([128, 2, 32], F32, tag="dnt", name="dnt")
            nc.vector.memset(dnt, 0.0)   # start=True resets the whole bank,
            # so the two sub-bank tails accumulate with start=False onto 0

            q3 = qsb.rearrange("p (r c) -> p r c", r=OWN)
            DRAIN_BY = 45
            nit = len(nxt_items)
            drained = 0
            j = 0
            for d1 in range(K):
                for d2 in range(K):
                    want = min(nit, nit * (j + 1) // DRAIN_BY + 1)
                    while drained < want:
                        nxt_items[drained]()
                        drained += 1
                    st = sp.tile([128, OWN, 56], FP16, tag="s", name="st")
                    nc.vector.tensor_tensor(
                        out=st,
                        in0=kbs[d1][:, :, d2:d2 + 56],
                        in1=q3,
                        op=mybir.AluOpType.mult)
                    et = ep.tile([128, NOWN], BF16, tag="e", name="et")
                    nc.scalar.activation(
                        out=et.rearrange("p (r c) -> p r c", r=OWN),
                        in_=st,
                        func=mybir.ActivationFunctionType.Exp,
                        bias=nbsb, scale=1.0)
                    tt = tp.tile([128, NOWN], BF16, tag="t", name="tt")
                    eng_t = nc.gpsimd if t_pool[j] else nc.vector
                    eng_t.tensor_tensor(
                        out=tt.rearrange("p (r c) -> p r c", r=OWN),
                        in0=et.rearrange("p (r c) -> p r c", r=OWN),
                        in1=vpad[:, d1:d1 + OWN, d2:d2 + 56],
                        op=mybir.AluOpType.mult)
                    first = (d1 == 0 and d2 == 0)
                    last = (d1 == K - 1 and d2 == K - 1)
                    for i, (c0, cw) in enumerate(slc):
                        nc.tensor.matmul(
                            den[:, i, :cw], lhsT=identsb,
                            rhs=et[:, c0:c0 + cw],
                            start=first, stop=last, skip_group_check=True)
                        nc.tensor.matmul(
                            num[:, i, :cw], lhsT=identsb,
                            rhs=tt[:, c0:c0 + cw],
                            start=first, stop=last, skip_group_check=True)
                    nc.tensor.matmul(
                        dnt[:, 0, :], lhsT=identsb, rhs=et[:, 1536:NOWN],
                        start=False, stop=last, skip_group_check=True)
                    nc.tensor.matmul(
                        dnt[:, 1, :], lhsT=identsb, rhs=tt[:, 1536:NOWN],
                        start=False, stop=last, skip_group_check=True)
                    j += 1
            while drained < nit:
                nxt_items[drained]()
                drained += 1

            rden = fin.tile([128, NOWN], F32, tag="rden", name="rden")
            outsb = fin.tile([128, NOWN], F32, tag="outsb", name="outsb")
            views = [(c0, cw, den[:, i, :cw], num[:, i, :cw])
                     for i, (c0, cw) in enumerate(slc)]
            views.append((1536, 32, dnt[:, 0, :], dnt[:, 1, :]))
            for c0, cw, dv, nv in views:
                sl = slice(c0, c0 + cw)
                nc.vector.reciprocal_approx_fast(out=rden[:, sl], in_=dv)
                nc.vector.tensor_tensor(out=outsb[:, sl], in0=nv,
                                        in1=rden[:, sl],
                                        op=mybir.AluOpType.mult)
            nc.sync.dma_start(out=out, in_=outsb)

        cur_tiles, cur_items = make_p1(0)
        for it in cur_items:
            it()
        for r in range(reps):
            if r + 1 < reps:
                nxt_tiles, nxt_items = make_p1(r + 1)
            else:
                nxt_tiles, nxt_items = None, []
            phase2(cur_tiles, nxt_items)
            cur_tiles = nxt_tiles

    nc.finalize()
    return nc


def _prep_inputs(x, w_q, w_k, w_v, rel_h, rel_w):
    """Build the 8 per-core input dicts (all host-side numpy)."""
    import ml_dtypes
    x4 = np.ascontiguousarray(np.asarray(x, np.float32).reshape(B, H, W, CIN))
    relh = np.asarray(rel_h, np.float32).reshape(128, K)
    relw = np.asarray(rel_w, np.float32).reshape(128, K)
    ws = [np.asarray(w, np.float32) for w in (w_q, w_k, w_v)]
    ident = np.eye(128, dtype=ml_dtypes.bfloat16)
    nbias = np.full((128, 1), SHIFT, np.float32)

    in_maps = []
    for core in range(8):
        chalf, b, shalf = core >> 2, (core >> 1) & 1, core & 1
        if chalf == 0:
            xm = x4[b]
            rel = relh
        else:
            xm = x4[b].transpose(1, 0, 2)
            rel = relw
        arr = np.zeros((PR, 56, CIN), np.float32)
        if shalf == 0:
            arr[PAD:PAD + SPAN] = xm[0:SPAN]
        else:
            arr[0:SPAN] = xm[H - SPAN:H]
        xt = np.ascontiguousarray(arr.reshape(NPOS, CIN).T)
        cs = slice(chalf * 128, chalf * 128 + 128)
        wt = np.ascontiguousarray(
            np.stack([w[cs].T for w in ws]))
        in_maps.append({"xt": xt, "wt": wt, "rel": np.ascontiguousarray(rel),
                        "ident": ident, "nbias": nbias})
    return in_maps


def _make_runner(nc, n_cores=8):
    bass2jax.install_neuronx_cc_hook()
    in_names, out_names, out_avals = [], [], []
    partition_name = (nc.partition_id_tensor.name
                      if nc.partition_id_tensor else None)
    for alloc in nc.m.functions[0].allocations:
        if not isinstance(alloc, mybir.MemoryLocationSet):
            continue
        name = alloc.memorylocations[0].name
        if alloc.kind == "ExternalInput":
            if name != partition_name:
                in_names.append(name)
        elif alloc.kind == "ExternalOutput":
            out_names.append(name)
            shape = tuple(alloc.tensor_shape)
            dtype = mybir.dt.np(alloc.dtype)
            out_avals.append(jax.core.ShapedArray(shape, dtype))
    n_params = len(in_names)
    n_outs = len(out_names)
    all_names = list(in_names) + out_names
    if partition_name is not None:
        all_names.append(partition_name)

    def _body(*args):
        operands = list(args)
        if partition_name is not None:
            operands.append(bass2jax.partition_id_tensor())
        outs = bass2jax._bass_exec_p.bind(
            *operands, out_avals=tuple(out_avals), in_names=tuple(all_names),
            out_names=tuple(out_names), lowering_input_output_aliases=(),
            sim_require_finite=True, sim_require_nnan=True, nc=nc)
        return tuple(outs)

    devices = jax.devices()[:n_cores]
    mesh = Mesh(np.asarray(devices), ("core",))
    donate = tuple(range(n_params, n_params + n_outs))
    sharded = jax.jit(
        shard_map(_body, mesh=mesh,
                  in_specs=(PartitionSpec("core"),) * (n_params + n_outs),
                  out_specs=(PartitionSpec("core"),) * n_outs,
                  check_rep=False),
        donate_argnums=donate, keep_unused=True)
    return sharded, in_names, out_names, out_avals


def _get_compiled(reps=1, **kw):
    key = ("runner", reps, tuple(sorted(kw.items())))
    if key not in _CACHE:
        nc = _build_nc(reps=reps, **kw)
        _CACHE[key] = _make_runner(nc)
    return _CACHE[key]


def make_device_args(in_maps, reps=1, **kw):
    _, in_names, _, _ = _get_compiled(reps, **kw)
    return [np.concatenate([np.asarray(m[nm]) for m in in_maps], axis=0)
            for nm in in_names]


def run_cores(concat_in, reps=1, **kw):
    sharded, in_names, out_names, out_avals = _get_compiled(reps, **kw)
    concat_zeros = [np.zeros((8 * a.shape[0], *a.shape[1:]), a.dtype)
                    for a in out_avals]
    outs = sharded(*concat_in, *concat_zeros)
    o = np.asarray(outs[out_names.index("out")]).reshape(8, 128, NOWN)
    return o


def _assemble(per_core_out):
    out4 = np.empty((B, CO, H, W), np.float32)
    for core in range(8):
        chalf, b, shalf = core >> 2, (core >> 1) & 1, core & 1
        blk = per_core_out[core].reshape(128, OWN, 56)
        lo = shalf * OWN
        if chalf == 0:
            out4[b, 0:128, lo:lo + OWN, :] = blk
        else:
            out4[b, 128:256, :, lo:lo + OWN] = blk.transpose(0, 2, 1)
    return out4.reshape(B, CO * H, W)


def kernel(x, w_q, w_k, w_v, rel_h, rel_w):
    in_maps = _prep_inputs(x, w_q, w_k, w_v, rel_h, rel_w)
    concat_in = make_device_args(in_maps)
    per_core = run_cores(concat_in)
    return _assemble(per_core)


# revision 11
# speedup vs baseline: 1.7127x; 1.0930x over previous
"""AttentionConv (7x7 windowed per-channel softmax attention) on 8 TRN2 cores.

Sharding: core = (chalf, batch, shalf); chalf=1 maps stored transposed so
rel_w folds like rel_h.  Per core: 128 channels x 28x56 positions.

Mixed-precision pipeline (vs the all-fp32 V1 at 230 us):
  Phase 1 (PE f32r, TF32-rounded inputs): q/k/v projections into fp32 PSUM;
    k stays fp32 (kpad), q -> fp16 (ACT copies), v -> fp16 (DVE copies).
  Phase 2, per window offset (d1, d2):
    kb[d1] = kpad rows + rel[:,d1] -> fp16   GpSimd broadcast-add, per d1
    s = kb_view * q                          fp16 TT on DVE (16-bit 2x mode)
    e = exp(s - 48) -> bf16                  ACT (the exp stream is the
                                             irreducible ~73 us/core floor)
    t = e * v -> bf16                        TT, 16/49 offsets on GpSimd
    den += I@e ; num += I@t                  PE bf16 identity matmuls into
                                             3x512 PSUM banks + a shared
                                             tail bank (sub-bank offsets,
                                             pre-zeroed, start=False - a
                                             start=True would reset the
                                             whole bank)
  out = num * reciprocal(den)                DVE
Cross-rep software pipelining: the next rep's projection groups + rel folds
are emitted interleaved into the current rep's phase-2 stream (DRAIN_BY),
with a static 8-bank PSUM layout (mm 1 + den 3 + num 3 + tails 1) so the
single mm bank hands off cleanly between reps.

Numerics: 9.4e-3 scale-relative absmax on HW (gate 2e-2): fp16 kb/s/q/v
rounding dominates (logit abs error ~|s|*2^-10.5), plus bf16 e/t.
Cost-model per-rep ~102 us; engines ACT/DVE/Pool/PE all at 77-84 us busy.
"""
import numpy as np
from contextlib import ExitStack

import jax
from jax.sharding import Mesh, PartitionSpec
from jax.experimental.shard_map import shard_map

import concourse.bass as bass
import concourse.bacc as bacc
import concourse.tile as tile
from concourse import mybir
from concourse import bass2jax

F32 = mybir.dt.float32
F32R = mybir.dt.float32r
BF16 = mybir.dt.bfloat16
FP16 = mybir.dt.float16

B, H, W, CIN, CO, K, PAD = 2, 56, 56, 512, 256, 7, 3
OWN = 28
SPAN = 31
PR = 34
PW = 62
NPOS = PR * 56      # 1904
NOWN = OWN * 56     # 1568
SHIFT = -48.0
NSL = 4
SLW = NOWN // NSL   # 392

_CACHE = {}
N_TP = 16           # t-mult offsets sent to GpSimd (of 49)
BUFS = 6
PROJ_F32R = True
V_FP16 = True
KB_ENG = "pool"     # engine for the per-d1 rel folds: pool | dve | act
NCH_DMA = 8         # x DMA position-chunks (each covers all 4 cin tiles)


def _build_nc(reps=1, n_tp=N_TP, proj_f32r=PROJ_F32R,
              v_fp16=V_FP16, bufs=BUFS, kb_eng=KB_ENG, nch=NCH_DMA):
    nc = bacc.Bacc("TRN2", target_bir_lowering=False, debug=False)
    XDT = F32R if proj_f32r else F32
    xt = nc.dram_tensor("xt", [CIN, NPOS], XDT, kind="ExternalInput").ap()
    wt = nc.dram_tensor("wt", [3, CIN, 128], XDT, kind="ExternalInput").ap()
    rel = nc.dram_tensor("rel", [128, K], F32, kind="ExternalInput").ap()
    ident = nc.dram_tensor("ident", [128, 128], BF16, kind="ExternalInput").ap()
    nbias = nc.dram_tensor("nbias", [128, 1], F32, kind="ExternalInput").ap()
    out = nc.dram_tensor("out", [128, NOWN], F32, kind="ExternalOutput").ap()

    VDT = FP16 if v_fp16 else BF16
    # matmul/recip col splits: 3x512 into den/num banks + 32-tail into the
    # shared dnt bank (sub-bank offsets for the two tails)
    slc = [(0, 512), (512, 512), (1024, 512)]

    # interleaved pool-assignment for the t-mult: n_tp of 49 offsets on
    # GpSimd; first/last kept on DVE so pipeline ends aren't on the slow
    # engine
    t_pool = [False] * 49
    acc = 0
    for j in range(1, 48):
        want = j * n_tp // 47
        if want > acc:
            t_pool[j] = True
            acc = want

    with tile.TileContext(nc) as tc, ExitStack() as ctx:
        per = ctx.enter_context(tc.tile_pool(name="per", bufs=1))
        ld = ctx.enter_context(tc.tile_pool(name="ld", bufs=1))

        wsb = ld.tile([128, 3, 4, 128], XDT)
        wtv = wt.rearrange("w (t p) m -> p w t m", p=128)
        nc.sync.dma_start(out=wsb[:, 1], in_=wtv[:, 1])   # k weights
        nc.sync.dma_start(out=wsb[:, 0], in_=wtv[:, 0])   # q weights
        relsb = per.tile([128, K], F32)
        nc.sync.dma_start(out=relsb, in_=rel)
        identsb = per.tile([128, 128], BF16)
        nc.sync.dma_start(out=identsb, in_=ident)
        nbsb = per.tile([128, 1], F32)
        nc.sync.dma_start(out=nbsb, in_=nbias)
        xsb = ld.tile([128, 4, NPOS], XDT)
        xtv = xt.rearrange("(t p) n -> p t n", p=128)
        chw = NPOS // nch
        for c in range(nch):
            if nch < 0:   # batched variant (t-dims in one DMA)
                nc.sync.dma_start(out=xsb[:, :, c * chw:(c + 1) * chw],
                                  in_=xtv[:, :, c * chw:(c + 1) * chw])
            else:
                for t in range(4):
                    nc.sync.dma_start(out=xsb[:, t, c * chw:(c + 1) * chw],
                                      in_=xtv[:, t, c * chw:(c + 1) * chw])
        nc.sync.dma_start(out=wsb[:, 2], in_=wtv[:, 2])   # v weights

        maps = ctx.enter_context(tc.tile_pool(name="maps", bufs=2))
        kbp = ctx.enter_context(tc.tile_pool(name="kbp", bufs=2))
        sp = ctx.enter_context(tc.tile_pool(name="sp", bufs=bufs))
        ep = ctx.enter_context(tc.tile_pool(name="ep", bufs=bufs))
        tp = ctx.enter_context(tc.tile_pool(name="tp", bufs=bufs))
        fin = ctx.enter_context(tc.tile_pool(name="fin", bufs=2))
        # static PSUM layout (8 banks total): mm 1 + den 3 + num 3 + dnt 1
        mm = ctx.enter_context(tc.tile_pool(name="mm", bufs=1, space="PSUM"))
        accp = ctx.enter_context(tc.tile_pool(name="acc", bufs=1,
                                              space="PSUM"))

        kv_slices = [(0, 7), (7, 7), (14, 7), (21, 7), (28, 6)]

        def make_p1(ri):
            """Allocate rep ri's map tiles and return (tiles, emit-closures).

            The closures are interleaved into the previous rep's phase-2
            emission so the projection matmuls fill PE gaps and the single
            mm PSUM bank never serializes back-to-back groups.
            """
            kpad = maps.tile([128, PR, PW], F32, tag="kpad", name=f"kpad{ri}")
            vpad = maps.tile([128, PR, PW], VDT, tag="vpad", name=f"vpad{ri}")
            qsb = maps.tile([128, NOWN], FP16, tag="qsb", name=f"qsb{ri}")
            kbs = [kbp.tile([128, OWN, PW], FP16, tag=f"kb{d1}",
                            name=f"kb{d1}_{ri}")
                   for d1 in range(K)]
            tiles = (kpad, vpad, qsb, kbs)

            def memsets():
                for buf in (kpad, vpad):
                    nc.gpsimd.memset(buf[:, :, 0:PAD], 0.0)
                    nc.gpsimd.memset(buf[:, :, PAD + 56:PW], 0.0)

            def proj_kv(wi, dst, r0, nr, eng_copy):
                pt = mm.tile([128, 392], F32, tag="mmkv", name="pt")
                n0, n1 = r0 * 56, (r0 + nr) * 56
                for t in range(4):
                    nc.tensor.matmul(pt[:, :nr * 56],
                                     lhsT=wsb[:, wi, t, :],
                                     rhs=xsb[:, t, n0:n1],
                                     start=(t == 0), stop=(t == 3))
                dstv = dst[:, r0:r0 + nr, PAD:PAD + 56]
                srcv = pt[:, :nr * 56].rearrange("p (r c) -> p r c", r=nr)
                if eng_copy == "act":
                    nc.scalar.copy(out=dstv, in_=srcv)
                elif eng_copy == "dma":
                    nc.sync.dma_start(out=dstv, in_=srcv)
                else:
                    nc.vector.tensor_copy(out=dstv, in_=srcv)

            def proj_q(i):
                pt = mm.tile([128, SLW], F32, tag="mmkv", name="pt")
                n0 = PAD * 56 + i * SLW
                for t in range(4):
                    nc.tensor.matmul(pt, lhsT=wsb[:, 0, t, :],
                                     rhs=xsb[:, t, n0:n0 + SLW],
                                     start=(t == 0), stop=(t == 3))
                nc.scalar.copy(out=qsb[:, i * SLW:(i + 1) * SLW], in_=pt)

            def build_kb(d1):
                dst, srcv = kbs[d1], kpad[:, d1:d1 + OWN, :]
                if kb_eng == "pool":
                    nc.gpsimd.tensor_tensor(
                        out=dst, in0=srcv,
                        in1=relsb[:, d1:d1 + 1].broadcast_to((128, OWN, PW)),
                        op=mybir.AluOpType.add)
                elif kb_eng == "act":
                    nc.scalar.activation(
                        out=dst, in_=srcv,
                        func=mybir.ActivationFunctionType.Identity,
                        bias=relsb[:, d1:d1 + 1], scale=1.0)
                else:
                    nc.vector.tensor_scalar_add(out=dst, in0=srcv,
                                                scalar1=relsb[:, d1:d1 + 1])

            from functools import partial
            items = [memsets]
            items += [partial(proj_kv, 1, kpad, r0, nr, "act")
                      for (r0, nr) in kv_slices]
            items += [partial(build_kb, d1) for d1 in range(K)]
            items += [partial(proj_q, i) for i in range(NSL)]
            items += [partial(proj_kv, 2, vpad, r0, nr, "dve")
                      for (r0, nr) in kv_slices]
            return tiles, items

        def phase2(tiles, nxt_items):
            """Emit rep's attention stream, draining nxt_items between
            offsets (all drained by offset DRAIN_BY so the next rep can
            start immediately)."""
            kpad, vpad, qsb, kbs = tiles
            den = accp.tile([128, 3, 512], F32, tag="den", name="den")
            num = accp.tile([128, 3, 512], F32, tag="num", name="num")
            dnt = accp.tile([128, 2, 32], F32, tag="dnt", name="dnt")
            nc.vector.memset(dnt, 0.0)   # start=True resets the whole bank,
            # so the two sub-bank tails accumulate with start=False onto 0

            q3 = qsb.rearrange("p (r c) -> p r c", r=OWN)
            DRAIN_BY = 45
            nit = len(nxt_items)
            drained = 0
            j = 0
            for d1 in range(K):
                for d2 in range(K):
                    want = min(nit, nit * (j + 1) // DRAIN_BY + 1)
                    while drained < want:
                        nxt_items[drained]()
                        drained += 1
                    st = sp.tile([128, OWN, 56], FP16, tag="s", name="st")
                    nc.vector.tensor_tensor(
                        out=st,
                        in0=kbs[d1][:, :, d2:d2 + 56],
                        in1=q3,
                        op=mybir.AluOpType.mult)
                    et = ep.tile([128, NOWN], BF16, tag="e", name="et")
                    nc.scalar.activation(
                        out=et.rearrange("p (r c) -> p r c", r=OWN),
                        in_=st,
                        func=mybir.ActivationFunctionType.Exp,
                        bias=nbsb, scale=1.0)
                    tt = tp.tile([128, NOWN], BF16, tag="t", name="tt")
                    eng_t = nc.gpsimd if t_pool[j] else nc.vector
                    eng_t.tensor_tensor(
                        out=tt.rearrange("p (r c) -> p r c", r=OWN),
                        in0=et.rearrange("p (r c) -> p r c", r=OWN),
                        in1=vpad[:, d1:d1 + OWN, d2:d2 + 56],
                        op=mybir.AluOpType.mult)
                    first = (d1 == 0 and d2 == 0)
                    last = (d1 == K - 1 and d2 == K - 1)
                    for i, (c0, cw) in enumerate(slc):
                        nc.tensor.matmul(
                            den[:, i, :cw], lhsT=identsb,
                            rhs=et[:, c0:c0 + cw],
                            start=first, stop=last, skip_group_check=True)
                        nc.tensor.matmul(
                            num[:, i, :cw], lhsT=identsb,
                            rhs=tt[:, c0:c0 + cw],
                            start=first, stop=last, skip_group_check=True)
                    nc.tensor.matmul(
                        dnt[:, 0, :], lhsT=identsb, rhs=et[:, 1536:NOWN],
                        start=False, stop=last, skip_group_check=True)
                    nc.tensor.matmul(
                        dnt[:, 1, :], lhsT=identsb, rhs=tt[:, 1536:NOWN],
                        start=False, stop=last, skip_group_check=True)
                    j += 1
            while drained < nit:
                nxt_items[drained]()
                drained += 1

            rden = fin.tile([128, NOWN], F32, tag="rden", name="rden")
            outsb = fin.tile([128, NOWN], F32, tag="outsb", name="outsb")
            views = [(c0, cw, den[:, i, :cw], num[:, i, :cw])
                     for i, (c0, cw) in enumerate(slc)]
            views.append((1536, 32, dnt[:, 0, :], dnt[:, 1, :]))
            for c0, cw, dv, nv in views:
                sl = slice(c0, c0 + cw)
                nc.vector.reciprocal_approx_fast(out=rden[:, sl], in_=dv)
                nc.vector.tensor_tensor(out=outsb[:, sl], in0=nv,
                                        in1=rden[:, sl],
                                        op=mybir.AluOpType.mult)
            nc.sync.dma_start(out=out, in_=outsb)

        cur_tiles, cur_items = make_p1(0)
        for it in cur_items:
            it()
        for r in range(reps):
            if r + 1 < reps:
                nxt_tiles, nxt_items = make_p1(r + 1)
            else:
                nxt_tiles, nxt_items = None, []
            phase2(cur_tiles, nxt_items)
            cur_tiles = nxt_tiles

    nc.finalize()
    return nc


def _prep_inputs(x, w_q, w_k, w_v, rel_h, rel_w):
    """Build the 8 per-core input dicts (all host-side numpy)."""
    import ml_dtypes
    x4 = np.ascontiguousarray(np.asarray(x, np.float32).reshape(B, H, W, CIN))
    relh = np.asarray(rel_h, np.float32).reshape(128, K)
    relw = np.asarray(rel_w, np.float32).reshape(128, K)
    ws = [np.asarray(w, np.float32) for w in (w_q, w_k, w_v)]
    ident = np.eye(128, dtype=ml_dtypes.bfloat16)
    nbias = np.full((128, 1), SHIFT, np.float32)

    in_maps = []
    for core in range(8):
        chalf, b, shalf = core >> 2, (core >> 1) & 1, core & 1
        if chalf == 0:
            xm = x4[b]
            rel = relh
        else:
            xm = x4[b].transpose(1, 0, 2)
            rel = relw
        arr = np.zeros((PR, 56, CIN), np.float32)
        if shalf == 0:
            arr[PAD:PAD + SPAN] = xm[0:SPAN]
        else:
            arr[0:SPAN] = xm[H - SPAN:H]
        xt = np.ascontiguousarray(arr.reshape(NPOS, CIN).T)
        cs = slice(chalf * 128, chalf * 128 + 128)
        wt = np.ascontiguousarray(
            np.stack([w[cs].T for w in ws]))
        in_maps.append({"xt": xt, "wt": wt, "rel": np.ascontiguousarray(rel),
                        "ident": ident, "nbias": nbias})
    return in_maps


def _make_runner(nc, n_cores=8):
    bass2jax.install_neuronx_cc_hook()
    in_names, out_names, out_avals = [], [], []
    partition_name = (nc.partition_id_tensor.name
                      if nc.partition_id_tensor else None)
    for alloc in nc.m.functions[0].allocations:
        if not isinstance(alloc, mybir.MemoryLocationSet):
            continue
        name = alloc.memorylocations[0].name
        if alloc.kind == "ExternalInput":
            if name != partition_name:
                in_names.append(name)
        elif alloc.kind == "ExternalOutput":
            out_names.append(name)
            shape = tuple(alloc.tensor_shape)
            dtype = mybir.dt.np(alloc.dtype)
            out_avals.append(jax.core.ShapedArray(shape, dtype))
    n_params = len(in_names)
    n_outs = len(out_names)
    all_names = list(in_names) + out_names
    if partition_name is not None:
        all_names.append(partition_name)

    def _body(*args):
        operands = list(args)
        if partition_name is not None:
            operands.append(bass2jax.partition_id_tensor())
        outs = bass2jax._bass_exec_p.bind(
            *operands, out_avals=tuple(out_avals), in_names=tuple(all_names),
            out_names=tuple(out_names), lowering_input_output_aliases=(),
            sim_require_finite=True, sim_require_nnan=True, nc=nc)
        return tuple(outs)

    devices = jax.devices()[:n_cores]
    mesh = Mesh(np.asarray(devices), ("core",))
    donate = tuple(range(n_params, n_params + n_outs))
    sharded = jax.jit(
        shard_map(_body, mesh=mesh,
                  in_specs=(PartitionSpec("core"),) * (n_params + n_outs),
                  out_specs=(PartitionSpec("core"),) * n_outs,
                  check_rep=False),
        donate_argnums=donate, keep_unused=True)
    return sharded, in_names, out_names, out_avals


def _get_compiled(reps=1, **kw):
    key = ("runner", reps, tuple(sorted(kw.items())))
    if key not in _CACHE:
        nc = _build_nc(reps=reps, **kw)
        _CACHE[key] = _make_runner(nc)
    return _CACHE[key]


def make_device_args(in_maps, reps=1, **kw):
    _, in_names, _, _ = _get_compiled(reps, **kw)
    return [np.concatenate([np.asarray(m[nm]) for m in in_maps], axis=0)
            for nm in in_names]


def run_cores(concat_in, reps=1, **kw):
    sharded, in_names, out_names, out_avals = _get_compiled(reps, **kw)
    concat_zeros = [np.zeros((8 * a.shape[0], *a.shape[1:]), a.dtype)
                    for a in out_avals]
    outs = sharded(*concat_in, *concat_zeros)
    o = np.asarray(outs[out_names.index("out")]).reshape(8, 128, NOWN)
    return o


def _assemble(per_core_out):
    out4 = np.empty((B, CO, H, W), np.float32)
    for core in range(8):
        chalf, b, shalf = core >> 2, (core >> 1) & 1, core & 1
        blk = per_core_out[core].reshape(128, OWN, 56)
        lo = shalf * OWN
        if chalf == 0:
            out4[b, 0:128, lo:lo + OWN, :] = blk
        else:
            out4[b, 128:256, :, lo:lo + OWN] = blk.transpose(0, 2, 1)
    return out4.reshape(B, CO * H, W)


def kernel(x, w_q, w_k, w_v, rel_h, rel_w):
    in_maps = _prep_inputs(x, w_q, w_k, w_v, rel_h, rel_w)
    concat_in = make_device_args(in_maps)
    per_core = run_cores(concat_in)
    return _assemble(per_core)


# revision 13
# speedup vs baseline: 3.9753x; 2.3210x over previous
"""AttentionConv (7x7 windowed per-channel softmax attention) on 8 TRN2 cores.

Sharding: core = (chalf, batch, shalf); chalf=1 maps stored transposed so
rel_w folds like rel_h.  Per core: 128 channels x 28x56 positions.

Mixed-precision pipeline (vs the all-fp32 V1 at 230 us):
  Phase 1 (PE f32r, TF32-rounded inputs): q/k/v projections into fp32 PSUM;
    k stays fp32 (kpad), q -> fp16 (ACT copies), v -> fp16 (DVE copies).
  Phase 2, per window offset (d1, d2):
    kb[d1] = kpad rows + rel[:,d1] -> fp16   GpSimd broadcast-add, per d1
    s = kb_view * q                          fp16 TT on DVE (16-bit 2x mode)
    e = exp(s - 48) -> bf16                  ACT (the exp stream is the
                                             irreducible ~73 us/core floor)
    t = e * v -> bf16                        TT, 16/49 offsets on GpSimd
    den += I@e ; num += I@t                  PE bf16 identity matmuls into
                                             3x512 PSUM banks + a shared
                                             tail bank (sub-bank offsets,
                                             pre-zeroed, start=False - a
                                             start=True would reset the
                                             whole bank)
  out = num * reciprocal(den)                DVE
Cross-rep software pipelining: the next rep's projection groups + rel folds
are emitted interleaved into the current rep's phase-2 stream (DRAIN_BY),
with a static 8-bank PSUM layout (mm 1 + den 3 + num 3 + tails 1) so the
single mm bank hands off cleanly between reps.

Numerics: 9.4e-3 scale-relative absmax on HW (gate 2e-2): fp16 kb/s/q/v
rounding dominates (logit abs error ~|s|*2^-10.5), plus bf16 e/t.
Cost-model per-rep ~102 us; engines ACT/DVE/Pool/PE all at 77-84 us busy.
"""
import numpy as np
from contextlib import ExitStack

import jax
from jax.sharding import Mesh, PartitionSpec
from jax.experimental.shard_map import shard_map

import concourse.bass as bass
import concourse.bacc as bacc
import concourse.tile as tile
from concourse import mybir
from concourse import bass2jax

F32 = mybir.dt.float32
F32R = mybir.dt.float32r
BF16 = mybir.dt.bfloat16
FP16 = mybir.dt.float16

B, H, W, CIN, CO, K, PAD = 2, 56, 56, 512, 256, 7, 3
OWN = 28
SPAN = 31
PR = 34
PW = 62
NPOS = PR * 56      # 1904
NOWN = OWN * 56     # 1568
SHIFT = -48.0
NSL = 4
SLW = NOWN // NSL   # 392

_CACHE = {}
N_TP = 16           # t-mult offsets sent to GpSimd (of 49)
BUFS = 6
PROJ_F32R = True
V_FP16 = True
KB_ENG = "pool"     # engine for the per-d1 rel folds: pool | dve | act
NCH_DMA = 8         # x DMA position-chunks (each covers all 4 cin tiles)


def _build_nc(reps=1, n_tp=N_TP, proj_f32r=PROJ_F32R,
              v_fp16=V_FP16, bufs=BUFS, kb_eng=KB_ENG, nch=NCH_DMA):
    nc = bacc.Bacc("TRN2", target_bir_lowering=False, debug=False)
    XDT = F32R if proj_f32r else F32
    xt = nc.dram_tensor("xt", [CIN, NPOS], XDT, kind="ExternalInput").ap()
    wt = nc.dram_tensor("wt", [3, CIN, 128], XDT, kind="ExternalInput").ap()
    rel = nc.dram_tensor("rel", [128, K], F32, kind="ExternalInput").ap()
    ident = nc.dram_tensor("ident", [128, 128], BF16, kind="ExternalInput").ap()
    nbias = nc.dram_tensor("nbias", [128, 1], F32, kind="ExternalInput").ap()
    out = nc.dram_tensor("out", [128, NOWN], F32, kind="ExternalOutput").ap()

    VDT = FP16 if v_fp16 else BF16
    # matmul/recip col splits: 3x512 into den/num banks + 32-tail into the
    # shared dnt bank (sub-bank offsets for the two tails)
    slc = [(0, 512), (512, 512), (1024, 512)]

    # interleaved pool-assignment for the t-mult: n_tp of 49 offsets on
    # GpSimd; first/last kept on DVE so pipeline ends aren't on the slow
    # engine
    t_pool = [False] * 49
    acc = 0
    for j in range(1, 48):
        want = j * n_tp // 47
        if want > acc:
            t_pool[j] = True
            acc = want

    with tile.TileContext(nc) as tc, ExitStack() as ctx:
        per = ctx.enter_context(tc.tile_pool(name="per", bufs=1))
        ld = ctx.enter_context(tc.tile_pool(name="ld", bufs=1))

        wsb = ld.tile([128, 3, 4, 128], XDT)
        wtv = wt.rearrange("w (t p) m -> p w t m", p=128)
        nc.sync.dma_start(out=wsb[:, 1], in_=wtv[:, 1])   # k weights
        nc.sync.dma_start(out=wsb[:, 0], in_=wtv[:, 0])   # q weights
        relsb = per.tile([128, K], F32)
        nc.sync.dma_start(out=relsb, in_=rel)
        identsb = per.tile([128, 128], BF16)
        nc.sync.dma_start(out=identsb, in_=ident)
        nbsb = per.tile([128, 1], F32)
        nc.sync.dma_start(out=nbsb, in_=nbias)
        xsb = ld.tile([128, 4, NPOS], XDT)
        xtv = xt.rearrange("(t p) n -> p t n", p=128)
        chw = NPOS // nch
        for c in range(nch):
            if nch < 0:   # batched variant (t-dims in one DMA)
                nc.sync.dma_start(out=xsb[:, :, c * chw:(c + 1) * chw],
                                  in_=xtv[:, :, c * chw:(c + 1) * chw])
            else:
                for t in range(4):
                    nc.sync.dma_start(out=xsb[:, t, c * chw:(c + 1) * chw],
                                      in_=xtv[:, t, c * chw:(c + 1) * chw])
        nc.sync.dma_start(out=wsb[:, 2], in_=wtv[:, 2])   # v weights

        maps = ctx.enter_context(tc.tile_pool(name="maps", bufs=2))
        kbp = ctx.enter_context(tc.tile_pool(name="kbp", bufs=2))
        sp = ctx.enter_context(tc.tile_pool(name="sp", bufs=bufs))
        ep = ctx.enter_context(tc.tile_pool(name="ep", bufs=bufs))
        tp = ctx.enter_context(tc.tile_pool(name="tp", bufs=bufs))
        fin = ctx.enter_context(tc.tile_pool(name="fin", bufs=2))
        # static PSUM layout (8 banks total): mm 1 + den 3 + num 3 + dnt 1
        mm = ctx.enter_context(tc.tile_pool(name="mm", bufs=1, space="PSUM"))
        accp = ctx.enter_context(tc.tile_pool(name="acc", bufs=1,
                                              space="PSUM"))

        kv_slices = [(0, 7), (7, 7), (14, 7), (21, 7), (28, 6)]

        def make_p1(ri):
            """Allocate rep ri's map tiles and return (tiles, emit-closures).

            The closures are interleaved into the previous rep's phase-2
            emission so the projection matmuls fill PE gaps and the single
            mm PSUM bank never serializes back-to-back groups.
            """
            kpad = maps.tile([128, PR, PW], F32, tag="kpad", name=f"kpad{ri}")
            vpad = maps.tile([128, PR, PW], VDT, tag="vpad", name=f"vpad{ri}")
            qsb = maps.tile([128, NOWN], FP16, tag="qsb", name=f"qsb{ri}")
            kbs = [kbp.tile([128, OWN, PW], FP16, tag=f"kb{d1}",
                            name=f"kb{d1}_{ri}")
                   for d1 in range(K)]
            tiles = (kpad, vpad, qsb, kbs)

            def memsets():
                for buf in (kpad, vpad):
                    nc.gpsimd.memset(buf[:, :, 0:PAD], 0.0)
                    nc.gpsimd.memset(buf[:, :, PAD + 56:PW], 0.0)

            def proj_kv(wi, dst, r0, nr, eng_copy):
                pt = mm.tile([128, 392], F32, tag="mmkv", name="pt")
                n0, n1 = r0 * 56, (r0 + nr) * 56
                for t in range(4):
                    nc.tensor.matmul(pt[:, :nr * 56],
                                     lhsT=wsb[:, wi, t, :],
                                     rhs=xsb[:, t, n0:n1],
                                     start=(t == 0), stop=(t == 3))
                dstv = dst[:, r0:r0 + nr, PAD:PAD + 56]
                srcv = pt[:, :nr * 56].rearrange("p (r c) -> p r c", r=nr)
                if eng_copy == "act":
                    nc.scalar.copy(out=dstv, in_=srcv)
                elif eng_copy == "dma":
                    nc.sync.dma_start(out=dstv, in_=srcv)
                else:
                    nc.vector.tensor_copy(out=dstv, in_=srcv)

            def proj_q(i):
                pt = mm.tile([128, SLW], F32, tag="mmkv", name="pt")
                n0 = PAD * 56 + i * SLW
                for t in range(4):
                    nc.tensor.matmul(pt, lhsT=wsb[:, 0, t, :],
                                     rhs=xsb[:, t, n0:n0 + SLW],
                                     start=(t == 0), stop=(t == 3))
                nc.scalar.copy(out=qsb[:, i * SLW:(i + 1) * SLW], in_=pt)

            def build_kb(d1):
                dst, srcv = kbs[d1], kpad[:, d1:d1 + OWN, :]
                if kb_eng == "pool":
                    nc.gpsimd.tensor_tensor(
                        out=dst, in0=srcv,
                        in1=relsb[:, d1:d1 + 1].broadcast_to((128, OWN, PW)),
                        op=mybir.AluOpType.add)
                elif kb_eng == "act":
                    nc.scalar.activation(
                        out=dst, in_=srcv,
                        func=mybir.ActivationFunctionType.Identity,
                        bias=relsb[:, d1:d1 + 1], scale=1.0)
                else:
                    nc.vector.tensor_scalar_add(out=dst, in0=srcv,
                                                scalar1=relsb[:, d1:d1 + 1])

            from functools import partial
            items = [memsets]
            items += [partial(proj_kv, 1, kpad, r0, nr, "act")
                      for (r0, nr) in kv_slices]
            items += [partial(build_kb, d1) for d1 in range(K)]
            items += [partial(proj_q, i) for i in range(NSL)]
            items += [partial(proj_kv, 2, vpad, r0, nr, "dve")
                      for (r0, nr) in kv_slices]
            return tiles, items

        def phase2(tiles, nxt_items):
            """Emit rep's attention stream, draining nxt_items between
            offsets (all drained by offset DRAIN_BY so the next rep can
            start immediately)."""
            kpad, vpad, qsb, kbs = tiles
            den = accp.tile([128, 3, 512], F32, tag="den", name="den")
            num = accp.tile([128, 3, 512], F32, tag="num", name="num")
            dnt = accp.tile([128, 2, 32], F32, tag="dnt", name="dnt")
            nc.vector.memset(dnt, 0.0)   # start=True resets the whole bank,
            # so the two sub-bank tails accumulate with start=False onto 0

            q3 = qsb.rearrange("p (r c) -> p r c", r=OWN)
            DRAIN_BY = 45
            nit = len(nxt_items)
            drained = 0
            j = 0
            for d1 in range(K):
                for d2 in range(K):
                    want = min(nit, nit * (j + 1) // DRAIN_BY + 1)
                    while drained < want:
                        nxt_items[drained]()
                        drained += 1
                    st = sp.tile([128, OWN, 56], FP16, tag="s", name="st")
                    nc.vector.tensor_tensor(
                        out=st,
                        in0=kbs[d1][:, :, d2:d2 + 56],
                        in1=q3,
                        op=mybir.AluOpType.mult)
                    et = ep.tile([128, NOWN], BF16, tag="e", name="et")
                    nc.scalar.activation(
                        out=et.rearrange("p (r c) -> p r c", r=OWN),
                        in_=st,
                        func=mybir.ActivationFunctionType.Exp,
                        bias=nbsb, scale=1.0)
                    tt = tp.tile([128, NOWN], BF16, tag="t", name="tt")
                    eng_t = nc.gpsimd if t_pool[j] else nc.vector
                    eng_t.tensor_tensor(
                        out=tt.rearrange("p (r c) -> p r c", r=OWN),
                        in0=et.rearrange("p (r c) -> p r c", r=OWN),
                        in1=vpad[:, d1:d1 + OWN, d2:d2 + 56],
                        op=mybir.AluOpType.mult)
                    first = (d1 == 0 and d2 == 0)
                    last = (d1 == K - 1 and d2 == K - 1)
                    for i, (c0, cw) in enumerate(slc):
                        nc.tensor.matmul(
                            den[:, i, :cw], lhsT=identsb,
                            rhs=et[:, c0:c0 + cw],
                            start=first, stop=last, skip_group_check=True)
                        nc.tensor.matmul(
                            num[:, i, :cw], lhsT=identsb,
                            rhs=tt[:, c0:c0 + cw],
                            start=first, stop=last, skip_group_check=True)
                    nc.tensor.matmul(
                        dnt[:, 0, :], lhsT=identsb, rhs=et[:, 1536:NOWN],
                        start=False, stop=last, skip_group_check=True)
                    nc.tensor.matmul(
                        dnt[:, 1, :], lhsT=identsb, rhs=tt[:, 1536:NOWN],
                        start=False, stop=last, skip_group_check=True)
                    j += 1
            while drained < nit:
                nxt_items[drained]()
                drained += 1

            rden = fin.tile([128, NOWN], F32, tag="rden", name="rden")
            outsb = fin.tile([128, NOWN], F32, tag="outsb", name="outsb")
            views = [(c0, cw, den[:, i, :cw], num[:, i, :cw])
                     for i, (c0, cw) in enumerate(slc)]
            views.append((1536, 32, dnt[:, 0, :], dnt[:, 1, :]))
            for c0, cw, dv, nv in views:
                sl = slice(c0, c0 + cw)
                nc.vector.reciprocal_approx_fast(out=rden[:, sl], in_=dv)
                nc.vector.tensor_tensor(out=outsb[:, sl], in0=nv,
                                        in1=rden[:, sl],
                                        op=mybir.AluOpType.mult)
            nc.sync.dma_start(out=out, in_=outsb)

        cur_tiles, cur_items = make_p1(0)
        for it in cur_items:
            it()
        for r in range(reps):
            if r + 1 < reps:
                nxt_tiles, nxt_items = make_p1(r + 1)
            else:
                nxt_tiles, nxt_items = None, []
            phase2(cur_tiles, nxt_items)
            cur_tiles = nxt_tiles

    nc.finalize()
    return nc


def _prep_inputs(x, w_q, w_k, w_v, rel_h, rel_w):
    """Build the 8 per-core input dicts (all host-side numpy)."""
    import ml_dtypes
    x4 = np.ascontiguousarray(np.asarray(x, np.float32).reshape(B, H, W, CIN))
    relh = np.asarray(rel_h, np.float32).reshape(128, K)
    relw = np.asarray(rel_w, np.float32).reshape(128, K)
    ws = [np.asarray(w, np.float32) for w in (w_q, w_k, w_v)]
    ident = np.eye(128, dtype=ml_dtypes.bfloat16)
    nbias = np.full((128, 1), SHIFT, np.float32)

    in_maps = []
    for core in range(8):
        chalf, b, shalf = core >> 2, (core >> 1) & 1, core & 1
        if chalf == 0:
            xm = x4[b]
            rel = relh
        else:
            xm = x4[b].transpose(1, 0, 2)
            rel = relw
        arr = np.zeros((PR, 56, CIN), np.float32)
        if shalf == 0:
            arr[PAD:PAD + SPAN] = xm[0:SPAN]
        else:
            arr[0:SPAN] = xm[H - SPAN:H]
        xt = np.ascontiguousarray(arr.reshape(NPOS, CIN).T)
        cs = slice(chalf * 128, chalf * 128 + 128)
        wt = np.ascontiguousarray(
            np.stack([w[cs].T for w in ws]))
        in_maps.append({"xt": xt, "wt": wt, "rel": np.ascontiguousarray(rel),
                        "ident": ident, "nbias": nbias})
    return in_maps


def _make_runner(nc, n_cores=8):
    bass2jax.install_neuronx_cc_hook()
    in_names, out_names, out_avals = [], [], []
    partition_name = (nc.partition_id_tensor.name
                      if nc.partition_id_tensor else None)
    for alloc in nc.m.functions[0].allocations:
        if not isinstance(alloc, mybir.MemoryLocationSet):
            continue
        name = alloc.memorylocations[0].name
        if alloc.kind == "ExternalInput":
            if name != partition_name:
                in_names.append(name)
        elif alloc.kind == "ExternalOutput":
            out_names.append(name)
            shape = tuple(alloc.tensor_shape)
            dtype = mybir.dt.np(alloc.dtype)
            out_avals.append(jax.core.ShapedArray(shape, dtype))
    n_params = len(in_names)
    n_outs = len(out_names)
    all_names = list(in_names) + out_names
    if partition_name is not None:
        all_names.append(partition_name)

    def _body(*args):
        operands = list(args)
        if partition_name is not None:
            operands.append(bass2jax.partition_id_tensor())
        outs = bass2jax._bass_exec_p.bind(
            *operands, out_avals=tuple(out_avals), in_names=tuple(all_names),
            out_names=tuple(out_names), lowering_input_output_aliases=(),
            sim_require_finite=True, sim_require_nnan=True, nc=nc)
        return tuple(outs)

    devices = jax.devices()[:n_cores]
    mesh = Mesh(np.asarray(devices), ("core",))
    donate = tuple(range(n_params, n_params + n_outs))
    sharded = jax.jit(
        shard_map(_body, mesh=mesh,
                  in_specs=(PartitionSpec("core"),) * (n_params + n_outs),
                  out_specs=(PartitionSpec("core"),) * n_outs,
                  check_rep=False),
        donate_argnums=donate, keep_unused=True)
    return sharded, in_names, out_names, out_avals


def _get_compiled(reps=1, **kw):
    key = ("runner", reps, tuple(sorted(kw.items())))
    if key not in _CACHE:
        nc = _build_nc(reps=reps, **kw)
        _CACHE[key] = _make_runner(nc)
    return _CACHE[key]


def make_device_args(in_maps, reps=1, **kw):
    _, in_names, _, _ = _get_compiled(reps, **kw)
    return [np.concatenate([np.asarray(m[nm]) for m in in_maps], axis=0)
            for nm in in_names]


def run_cores(concat_in, reps=1, **kw):
    sharded, in_names, out_names, out_avals = _get_compiled(reps, **kw)
    concat_zeros = [np.zeros((8 * a.shape[0], *a.shape[1:]), a.dtype)
                    for a in out_avals]
    outs = sharded(*concat_in, *concat_zeros)
    o = np.asarray(outs[out_names.index("out")]).reshape(8, 128, NOWN)
    return o


def _assemble(per_core_out):
    out4 = np.empty((B, CO, H, W), np.float32)
    for core in range(8):
        chalf, b, shalf = core >> 2, (core >> 1) & 1, core & 1
        blk = per_core_out[core].reshape(128, OWN, 56)
        lo = shalf * OWN
        if chalf == 0:
            out4[b, 0:128, lo:lo + OWN, :] = blk
        else:
            out4[b, 128:256, :, lo:lo + OWN] = blk.transpose(0, 2, 1)
    return out4.reshape(B, CO * H, W)


def kernel(x, w_q, w_k, w_v, rel_h, rel_w):
    in_maps = _prep_inputs(x, w_q, w_k, w_v, rel_h, rel_w)
    concat_in = make_device_args(in_maps)
    per_core = run_cores(concat_in)
    return _assemble(per_core)
